# revision 9
# baseline (speedup 1.0000x reference)
"""Trainium2 Bass kernel for the CliffordKAN layer problem.

Math (see reference):
  rbf[b,i,g]  = exp(-|x[b,i,:] - grid[g,:]|^2)
  out[b,o,x]  = sum_{i,g} rbf[b,i,g] * weights[i,o,g,x]
              + sum_{i,y} silu(x)[b,i,y] * M2[i,y,o,x] + sum_i silu_bias[i,o,x]
  where M2[i,y,o,z] = sum_x silu_weight[i,o,x] * C[x,y,z]  (Cayley tensor)

Everything collapses into ONE accumulation into PSUM[b, (o,x)] with
contraction index k = (i, g) of size 64*512 = 32768 per core (plus 384
silu rows).  Sharding: grid dimension G=4096 split across 8 cores
(512 grid points / 33.5 MB of weights per core); host sums the 8
partial (64, 256) outputs.

Per-core device program:
  - rbf argument -|x-g|^2 via an augmented K=6 matmul:
      lhsT = [2*g_0..2*g_3, -|g|^2, 1]  (6, 128 g-block)   stationary
      rhs  = [x_0..x_3, 1, -|x|^2]      (6, 512 (i,b)-cols) moving
    -> PSUM (128, 512), evicted through ScalarE Exp into SBUF in
    exactly the ((i,g), b) layout the big matmul wants as stationary.
  - big contraction: 256 float32r matmuls (full-rate fp32, N=256)
    accumulating into one PSUM tile, W streamed from HBM in 2 MB DMAs.
  - silu branch: 3 extra fp32 matmuls from host-prepped tensors
    (values nonzero only on core 0).
"""

import numpy as np

from concourse import bacc, bass, mybir  # noqa: F401  (bass kept for spacing APIs)
from concourse.bass_utils import run_bass_kernel_spmd
from concourse.tile import TileContext

B, I, O, G, X = 64, 64, 64, 4096, 4
NCORES = 8
GS = G // NCORES            # grid points per core = 512
NGB = GS // 128             # g-blocks per core = 4
NKT = NGB * I               # big-matmul k-tiles per core = 256
TPB = 16                    # k-tiles per DMA batch (= 2 MB)
NBLK = NKT // TPB           # 16 weight DMA batches
OX = O * X                  # 256
IB = I * B                  # 4096
NCH = IB // 512             # rbf chunks (N=512 matmuls) per g-block = 8
IPC = 512 // B              # i's per rbf chunk = 8

_nc_cache = None
last_results = None         # test harness reads exec_time_ns off this


def _cayley():
    C = np.zeros((4, 4, 4), dtype=np.float32)
    entries = [
        (0, 0, 0, 1), (0, 1, 1, 1), (0, 2, 2, 1), (0, 3, 3, 1),
        (1, 0, 1, 1), (1, 1, 0, 1), (1, 2, 3, 1), (1, 3, 2, 1),
        (2, 0, 2, 1), (2, 1, 3, -1), (2, 2, 0, 1), (2, 3, 1, -1),
        (3, 0, 3, 1), (3, 1, 2, -1), (3, 2, 1, 1), (3, 3, 0, -1),
    ]
    for xx, y, z, s in entries:
        C[xx, y, z] = s
    return C


def _build_bass():
    global _nc_cache
    if _nc_cache is not None:
        return _nc_cache

    nc = bacc.Bacc(
        "TRN2", target_bir_lowering=False, debug=False, num_devices=NCORES
    )
    f32 = mybir.dt.float32
    f32r = mybir.dt.float32r

    wt = nc.dram_tensor("wt", [NBLK, 128, TPB, OX], f32r, kind="ExternalInput")
    ga = nc.dram_tensor("ga", [6, GS], f32, kind="ExternalInput")
    xa = nc.dram_tensor("xa", [6, IB], f32, kind="ExternalInput")
    ls = nc.dram_tensor("ls", [128, 3, B], f32, kind="ExternalInput")
    ws = nc.dram_tensor("ws", [128, 3, OX], f32, kind="ExternalInput")
    out = nc.dram_tensor("out", [B, OX], f32, kind="ExternalOutput")

    with TileContext(nc) as tc:
        with (
            tc.tile_pool(name="const", bufs=1) as const,
            tc.tile_pool(name="wpool", bufs=4) as wpool,
            tc.tile_pool(name="rpool", bufs=3) as rpool,
            tc.tile_pool(name="psa", bufs=2, space="PSUM") as psa_pool,
            tc.tile_pool(name="pso", bufs=1, space="PSUM") as pso_pool,
        ):
            ga_t = const.tile([6, GS], f32)
            nc.sync.dma_start(ga_t[:], ga[:])
            xa_t = const.tile([6, IB], f32)
            nc.sync.dma_start(xa_t[:], xa[:])
            ls_t = const.tile([128, 3, B], f32)
            nc.sync.dma_start(ls_t[:], ls[:])
            ws_t = const.tile([128, 3, OX], f32)
            nc.sync.dma_start(ws_t[:], ws[:])

            pso = pso_pool.tile([B, OX], f32)

            q = 0  # big-matmul k-tile counter, in W stream order
            for gb in range(NGB):
                for nb in range(NCH):
                    psa = psa_pool.tile([128, 512], f32)
                    nc.tensor.matmul(
                        psa[:],
                        ga_t[:, gb * 128:(gb + 1) * 128],
                        xa_t[:, nb * 512:(nb + 1) * 512],
                        start=True,
                        stop=True,
                    )
                    rbf = rpool.tile([128, 512], f32r)
                    nc.scalar.activation(
                        rbf[:], psa[:], mybir.ActivationFunctionType.Exp
                    )
                    for il in range(IPC):
                        blk, t = divmod(q, TPB)
                        if t == 0:
                            w_t = wpool.tile([128, TPB, OX], f32r)
                            nc.sync.dma_start(w_t[:], wt[blk])
                        nc.tensor.matmul(
                            pso[:],
                            rbf[:, il * B:(il + 1) * B],
                            w_t[:, t, :],
                            start=(q == 0),
                            stop=False,
                            skip_group_check=True,
                        )
                        q += 1
            for s in range(3):
                nc.tensor.matmul(
                    pso[:],
                    ls_t[:, s, :],
                    ws_t[:, s, :],
                    start=False,
                    stop=(s == 2),
                    skip_group_check=True,
                )
            out_t = const.tile([B, OX], f32)
            nc.vector.tensor_copy(out_t[:], pso[:])
            nc.sync.dma_start(out[:], out_t[:])

    nc.compile()
    _nc_cache = nc
    return nc


def make_core_inputs(x, grid, weights, silu_weight, silu_bias):
    """Host-side shard + layout prep. Returns list of 8 input dicts."""
    x = np.ascontiguousarray(x, dtype=np.float32)
    grid = np.ascontiguousarray(grid, dtype=np.float32)
    weights = np.ascontiguousarray(weights, dtype=np.float32)
    silu_weight = np.ascontiguousarray(silu_weight, dtype=np.float32)
    silu_bias = np.ascontiguousarray(silu_bias, dtype=np.float32)

    # Pre-round W to fp32r (the PE's fast-fp32 format keeps only the top
    # 11 mantissa bits; the BIR verifier requires fp32r matmul operands to
    # be produced already rounded).
    def to_fp32r(a):
        bits = a.view(np.uint32)
        r = bits + 0x7FF + ((bits >> 12) & 1)
        return (r & np.uint32(0xFFFFF000)).view(np.float32)

    # xa: (6, I*B), column j = i*B + b
    xt = x.transpose(1, 0, 2)                       # (I, B, X)
    xa = np.empty((6, IB), dtype=np.float32)
    xa[0:4] = xt.reshape(IB, X).T
    xa[4] = 1.0
    xa[5] = -(xt ** 2).sum(-1).reshape(IB)

    # silu lhsT (core 0 only): rows k2 = i*4+y -> silu(x)[b,i,y]; row 256 -> 1
    sx = x / (1.0 + np.exp(-x))                     # silu(x), (B, I, X)
    lsf = np.zeros((384, B), dtype=np.float32)
    lsf[0:256] = sx.transpose(1, 2, 0).reshape(256, B)
    lsf[256] = 1.0
    ls0 = np.ascontiguousarray(lsf.reshape(3, 128, B).transpose(1, 0, 2))
    lsz = np.zeros_like(ls0)

    # silu rhs: M2[(i,y),(o,z)] = sum_x silu_weight[i,o,x]*C[x,y,z]; row 256 bias
    C = _cayley()
    m2 = np.einsum("iox,xyz->iyoz", silu_weight, C).reshape(256, OX)
    wsf = np.zeros((384, OX), dtype=np.float32)
    wsf[0:256] = m2
    wsf[256] = silu_bias.sum(axis=0).reshape(OX)
    ws = np.ascontiguousarray(wsf.reshape(3, 128, OX).transpose(1, 0, 2))

    in_maps = []
    for c in range(NCORES):
        gsl = slice(c * GS, (c + 1) * GS)
        gc = grid[gsl]                              # (GS, 4)
        ga = np.empty((6, GS), dtype=np.float32)
        ga[0:4] = 2.0 * gc.T
        ga[4] = -(gc ** 2).sum(-1)
        ga[5] = 1.0

        # W slab -> [blk, p, t, c] with k-tile q = gb*I + i, rows p = g in block
        warr = weights[:, :, gsl, :].transpose(0, 2, 1, 3).reshape(I, GS, OX)
        tmp = warr.reshape(I, NGB, 128, OX).transpose(1, 0, 2, 3)
        tmp = tmp.reshape(NBLK, TPB, 128, OX).transpose(0, 2, 1, 3)
        wt = to_fp32r(np.ascontiguousarray(tmp))

        in_maps.append({
            "wt": wt,
            "ga": np.ascontiguousarray(ga),
            "xa": xa,
            "ls": ls0 if c == 0 else lsz,
            "ws": ws,
        })
    return in_maps


def kernel(x, grid, weights, silu_weight, silu_bias):
    global last_results
    nc = _build_bass()
    in_maps = make_core_inputs(x, grid, weights, silu_weight, silu_bias)
    res = run_bass_kernel_spmd(nc, in_maps, list(range(NCORES)))
    last_results = res
    acc = np.zeros((B, OX), dtype=np.float32)
    for r in res.results:
        acc += r["out"]
    return acc.reshape(B, O, X)


# revision 12
# speedup vs baseline: 51.9621x; 51.9621x over previous
"""Trainium2 Bass kernel for the CliffordKAN layer problem.

Math (see reference):
  rbf[b,i,g]  = exp(-|x[b,i,:] - grid[g,:]|^2)
  out[b,o,x]  = sum_{i,g} rbf[b,i,g] * weights[i,o,g,x]
              + sum_{i,y} silu(x)[b,i,y] * M2[i,y,o,x] + sum_i silu_bias[i,o,x]
  where M2[i,y,o,z] = sum_x silu_weight[i,o,x] * C[x,y,z]  (Cayley tensor)

Everything collapses into ONE accumulation into PSUM[b, (o,x)] with
contraction index k = (i, g) of size 64*512 = 32768 per core (plus 384
silu rows).  Sharding: grid dimension G=4096 split across 8 cores
(512 grid points / 33.5 MB of weights per core); host sums the 8
partial (64, 256) outputs.

Per-core device program:
  - rbf argument -|x-g|^2 via an augmented K=6 matmul:
      lhsT = [2*g_0..2*g_3, -|g|^2, 1]  (6, 128 g-block)   stationary
      rhs  = [x_0..x_3, 1, -|x|^2]      (6, 512 (i,b)-cols) moving
    -> PSUM (128, 512), evicted through ScalarE Exp into SBUF in
    exactly the ((i,g), b) layout the big matmul wants as stationary.
  - big contraction: 256 float32r matmuls (full-rate fp32, N=256)
    accumulating into one PSUM tile, W streamed from HBM in 2 MB DMAs.
  - silu branch: 3 extra fp32 matmuls from host-prepped tensors
    (values nonzero only on core 0).
"""

import numpy as np

from concourse import bacc, bass, mybir  # noqa: F401  (bass kept for spacing APIs)
from concourse.bass_utils import run_bass_kernel_spmd
from concourse.tile import TileContext

B, I, O, G, X = 64, 64, 64, 4096, 4
NCORES = 8
GS = G // NCORES            # grid points per core = 512
NGB = GS // 128             # g-blocks per core = 4
NKT = NGB * I               # big-matmul k-tiles per core = 256
TPB = 16                    # k-tiles per DMA batch (= 2 MB)
NBLK = NKT // TPB           # 16 weight DMA batches
OX = O * X                  # 256
IB = I * B                  # 4096
NCH = IB // 512             # rbf chunks (N=512 matmuls) per g-block = 8
IPC = 512 // B              # i's per rbf chunk = 8

_nc_cache = None
last_results = None         # test harness reads exec_time_ns off this


def _cayley():
    C = np.zeros((4, 4, 4), dtype=np.float32)
    entries = [
        (0, 0, 0, 1), (0, 1, 1, 1), (0, 2, 2, 1), (0, 3, 3, 1),
        (1, 0, 1, 1), (1, 1, 0, 1), (1, 2, 3, 1), (1, 3, 2, 1),
        (2, 0, 2, 1), (2, 1, 3, -1), (2, 2, 0, 1), (2, 3, 1, -1),
        (3, 0, 3, 1), (3, 1, 2, -1), (3, 2, 1, 1), (3, 3, 0, -1),
    ]
    for xx, y, z, s in entries:
        C[xx, y, z] = s
    return C


def _build_bass(reps=1):
    """Build the per-core program. reps>1 unrolls the whole body multiple
    times (same inputs/outputs) — used only for steady-state benchmarking."""
    global _nc_cache
    if reps == 1 and _nc_cache is not None:
        return _nc_cache

    nc = bacc.Bacc(
        "TRN2", target_bir_lowering=False, debug=False, num_devices=NCORES
    )
    f32 = mybir.dt.float32
    f32r = mybir.dt.float32r

    wt = nc.dram_tensor("wt", [NBLK, 128, TPB, OX], f32r, kind="ExternalInput")
    ga = nc.dram_tensor("ga", [6, GS], f32, kind="ExternalInput")
    xa = nc.dram_tensor("xa", [6, IB], f32, kind="ExternalInput")
    ls = nc.dram_tensor("ls", [128, 3, B], f32, kind="ExternalInput")
    ws = nc.dram_tensor("ws", [128, 3, OX], f32, kind="ExternalInput")
    out = nc.dram_tensor("out", [B, OX], f32, kind="ExternalOutput")

    with TileContext(nc) as tc:
        with (
            tc.tile_pool(name="const", bufs=1) as const,
            tc.tile_pool(name="wpool", bufs=4) as wpool,
            tc.tile_pool(name="rpool", bufs=3) as rpool,
            tc.tile_pool(name="psa", bufs=2, space="PSUM") as psa_pool,
            tc.tile_pool(name="pso", bufs=1, space="PSUM") as pso_pool,
        ):
            ga_t = const.tile([6, GS], f32)
            nc.sync.dma_start(ga_t[:], ga[:])
            xa_t = const.tile([6, IB], f32)
            nc.sync.dma_start(xa_t[:], xa[:])
            ls_t = const.tile([128, 3, B], f32)
            nc.sync.dma_start(ls_t[:], ls[:])
            ws_t = const.tile([128, 3, OX], f32)
            nc.sync.dma_start(ws_t[:], ws[:])

            pso = pso_pool.tile([B, OX], f32)

            for _rep in range(reps):
                q = 0  # big-matmul k-tile counter, in W stream order
                for gb in range(NGB):
                    for nb in range(NCH):
                        psa = psa_pool.tile([128, 512], f32)
                        nc.tensor.matmul(
                            psa[:],
                            ga_t[:, gb * 128:(gb + 1) * 128],
                            xa_t[:, nb * 512:(nb + 1) * 512],
                            start=True,
                            stop=True,
                        )
                        rbf = rpool.tile([128, 512], f32r)
                        nc.scalar.activation(
                            rbf[:], psa[:], mybir.ActivationFunctionType.Exp
                        )
                        for il in range(IPC):
                            blk, t = divmod(q, TPB)
                            if t == 0:
                                w_t = wpool.tile([128, TPB, OX], f32r)
                                nc.sync.dma_start(w_t[:], wt[blk])
                            nc.tensor.matmul(
                                pso[:],
                                rbf[:, il * B:(il + 1) * B],
                                w_t[:, t, :],
                                start=(q == 0),
                                stop=False,
                                skip_group_check=True,
                            )
                            q += 1
                for s in range(3):
                    nc.tensor.matmul(
                        pso[:],
                        ls_t[:, s, :],
                        ws_t[:, s, :],
                        start=False,
                        stop=(s == 2),
                        skip_group_check=True,
                    )
            out_t = const.tile([B, OX], f32)
            nc.vector.tensor_copy(out_t[:], pso[:])
            nc.sync.dma_start(out[:], out_t[:])

    nc.compile()
    _nc_cache = nc
    return nc


def make_core_inputs(x, grid, weights, silu_weight, silu_bias):
    """Host-side shard + layout prep. Returns list of 8 input dicts."""
    x = np.ascontiguousarray(x, dtype=np.float32)
    grid = np.ascontiguousarray(grid, dtype=np.float32)
    weights = np.ascontiguousarray(weights, dtype=np.float32)
    silu_weight = np.ascontiguousarray(silu_weight, dtype=np.float32)
    silu_bias = np.ascontiguousarray(silu_bias, dtype=np.float32)

    # Pre-round W to fp32r (the PE's fast-fp32 format keeps only the top
    # 11 mantissa bits; the BIR verifier requires fp32r matmul operands to
    # be produced already rounded).
    def to_fp32r(a):
        bits = a.view(np.uint32)
        r = bits + 0x7FF + ((bits >> 12) & 1)
        return (r & np.uint32(0xFFFFF000)).view(np.float32)

    # xa: (6, I*B), column j = i*B + b
    xt = x.transpose(1, 0, 2)                       # (I, B, X)
    xa = np.empty((6, IB), dtype=np.float32)
    xa[0:4] = xt.reshape(IB, X).T
    xa[4] = 1.0
    xa[5] = -(xt ** 2).sum(-1).reshape(IB)

    # silu lhsT (core 0 only): rows k2 = i*4+y -> silu(x)[b,i,y]; row 256 -> 1
    sx = x / (1.0 + np.exp(-x))                     # silu(x), (B, I, X)
    lsf = np.zeros((384, B), dtype=np.float32)
    lsf[0:256] = sx.transpose(1, 2, 0).reshape(256, B)
    lsf[256] = 1.0
    ls0 = np.ascontiguousarray(lsf.reshape(3, 128, B).transpose(1, 0, 2))
    lsz = np.zeros_like(ls0)

    # silu rhs: M2[(i,y),(o,z)] = sum_x silu_weight[i,o,x]*C[x,y,z]; row 256 bias
    C = _cayley()
    m2 = np.einsum("iox,xyz->iyoz", silu_weight, C).reshape(256, OX)
    wsf = np.zeros((384, OX), dtype=np.float32)
    wsf[0:256] = m2
    wsf[256] = silu_bias.sum(axis=0).reshape(OX)
    ws = np.ascontiguousarray(wsf.reshape(3, 128, OX).transpose(1, 0, 2))

    in_maps = []
    for c in range(NCORES):
        gsl = slice(c * GS, (c + 1) * GS)
        gc = grid[gsl]                              # (GS, 4)
        ga = np.empty((6, GS), dtype=np.float32)
        ga[0:4] = 2.0 * gc.T
        ga[4] = -(gc ** 2).sum(-1)
        ga[5] = 1.0

        # W slab -> [blk, p, t, c] with k-tile q = gb*I + i, rows p = g in block
        warr = weights[:, :, gsl, :].transpose(0, 2, 1, 3).reshape(I, GS, OX)
        tmp = warr.reshape(I, NGB, 128, OX).transpose(1, 0, 2, 3)
        tmp = tmp.reshape(NBLK, TPB, 128, OX).transpose(0, 2, 1, 3)
        wt = to_fp32r(np.ascontiguousarray(tmp))

        in_maps.append({
            "wt": wt,
            "ga": np.ascontiguousarray(ga),
            "xa": xa,
            "ls": ls0 if c == 0 else lsz,
            "ws": ws,
        })
    return in_maps


def kernel(x, grid, weights, silu_weight, silu_bias):
    global last_results
    nc = _build_bass()
    in_maps = make_core_inputs(x, grid, weights, silu_weight, silu_bias)
    res = run_bass_kernel_spmd(nc, in_maps, list(range(NCORES)))
    last_results = res
    acc = np.zeros((B, OX), dtype=np.float32)
    for r in res.results:
        acc += r["out"]
    return acc.reshape(B, O, X)
